# revision 8
# baseline (speedup 1.0000x reference)
"""Banded DTW (window=100) on Trainium2, 8 NeuronCores — truncated-DP version.

Problem: x, y of shape (T=1024, N=32, C=4). Per trace n: banded DTW on the
(1024, 1024) pairwise-distance grid, band j in [i-100, i+100); cells outside
the band hold 0 (torch quirk); row 0 / col 0 seeded with raw distances.
Output: scalar mean over the 32 per-trace DTW values.

Key optimization: the out-of-band zeros leak into the band at BOTH band edges
(acc[i, i+99] = d, and the row state re-enters at 0 on the left edge), so the
DP forgets its history: a monotone lower/upper-bound sandwich (init row i0
with 0s vs +BIG) shows the final cell is exact for any i0 <= 900. We run only
rows 900..1023 (124 rows instead of 1024), seeding row 900 with its raw
distance band — certified rel err ~1e-7 in fp64 (fp16 DP state was tried
and fails: DP values ~200-600 make fp16 rounding accumulate to 2.8e-2).

Layout (4 traces per core, data parallel over 8 cores):
  Band-relative storage u = j - (i - 100), u in [0, 200); column 200 is a
  never-written zero boundary slot (replaces the baseline's mask multiply).
  Row recurrence  cur[u] = min(min(prev[u], prev[u+1]), cur[u-1]) + d[u]
  = ONE tensor_tensor (min of shifted pair) + ONE tensor_tensor_scan
  (op0=min, op1=add) per row, all fp32,
  4 traces riding the partition dim. Phase A computes banded distances for
  all 4 traces at once on 124 partitions (4 traces x 31 rows per group) and
  DMA-relayouts each trace's rows into its DP partition. Startup hiding:
  input DMAs ride the SP ring, relayout DMAs the idle PE ring, xneg on
  GPSIMD, and a warmup Sqrt hoists the ACT table load; the first DP row
  reads the seed band straight out of dpband (no seed copy).
"""

import os
import sys

import numpy as np

for _p in ("/opt/trn_rl_repo", "/root/.axon_site/_ro/trn_rl_repo"):
    if os.path.isdir(_p) and _p not in sys.path:
        sys.path.insert(0, _p)

import concourse.bass as bass
import concourse.bacc as bacc
import concourse.mybir as mybir
from concourse.bass_utils import run_bass_kernel_spmd
from concourse.tile import TileContext

T = 1024          # time steps (both sequences)
C = 4             # channels
N = 32            # traces
NCORES = 8
TPC = N // NCORES  # 4 traces per core
WIN = 100
I0 = 900           # first DP row (certified: any i0 <= 900 is exact)
K = T - I0         # 124 DP rows
RW = 2 * WIN       # 200 real band cells per row, u in [0, 200)
SW = RW + 1        # row stride: +1 zero boundary slot (u=200)
GR = 31            # phase-A rows per group (4 traces x 31 rows = 124 parts)
NG = K // GR       # 4 groups
J0 = I0 - WIN      # 800: first y index needed
YL = 324           # y slice length: j in [800, 1124), zero-padded past 1023

F32 = mybir.dt.float32
F16 = mybir.dt.float16
AF = mybir.ActivationFunctionType
OP = mybir.AluOpType

_CACHE = {}


def _build_nc():
    # Bacc (not raw Bass): its compile() pass splits multi-wait sync infos —
    # the TRN2 ISA allows at most one sync wait per instruction.
    nc = bacc.Bacc()
    # x pre-arranged on host to the phase-A layout: [t*GR+r, g*C+c]
    x = nc.declare_dram_parameter("x", [TPC * GR, NG * C], F32, isOutput=False)
    ypd = nc.declare_dram_parameter("ypd", [TPC, C, YL], F32, isOutput=False)
    out = nc.declare_dram_parameter("out", [TPC, 1], F32, isOutput=True)

    with TileContext(nc) as tc:
        with (
            tc.tile_pool(name="pa", bufs=2) as pa,
            tc.tile_pool(name="dp", bufs=1) as dp,
        ):
            # warmup: force the Square/Sqrt ACT table load before any data
            # lands, off the group-0 critical path.
            warm = dp.tile([1, 1], F32)
            nc.gpsimd.memset(warm[:], 1.0)
            nc.scalar.activation(warm[:], warm[:], AF.Sqrt)

            # DP-state tiles + memsets early so the Pool queue clears them
            # while inputs stream in.
            dpband = dp.tile([TPC, K, SW], F32)
            # zero the boundary column (u=200): read as prev[u+1] at u=199
            # and as the i=924 row's prev[200]; never written afterwards.
            nc.gpsimd.memset(dpband[0:TPC, 0:K, RW:SW], 0.0)
            prev = dp.tile([TPC, SW], F32)
            cur = dp.tile([TPC, SW], F32)
            m = dp.tile([TPC, SW], F32)
            nc.gpsimd.memset(m[:], 0.0)    # m[199] stays 0 for full rows
            nc.gpsimd.memset(prev[:], 0.0)
            nc.gpsimd.memset(cur[:], 0.0)  # cur[200] stays 0 forever

            # x for all groups in one contiguous DMA (host pre-arranged)
            xall = pa.tile([TPC * GR, NG * C], F32, tag="xall")
            nc.sync.dma_start(xall[:], x[:, :])
            xneg = pa.tile([TPC * GR, NG * C], F32, tag="xneg")
            nc.gpsimd.tensor_scalar_mul(xneg[:], xall[:], -1.0)

            # ---------------- Phase A: banded distances -----------------
            # group g covers rows I0+g*GR .. +GR of ALL 4 traces:
            # partition p = t*GR + r. D[p][u] = ||x[t,row] - y[row-100+u]||.
            # sq_c = (y_c - x_c)^2 via ACT Square with per-partition bias
            # (exact); adds on GPSIMD; DVE stays free for the DP chain.
            # All input DMAs issue upfront on the SP ring (in-order queue:
            # nothing may block it); relayouts ride the ACT ring right after
            # each sqrt so group 0 reaches the DP with no cross-engine hop.
            ydalls = []
            for g in range(NG):
                # ydall[t*GR+r, c*RW+u] = ypd[t, c, g*GR + r + u]
                ydall = pa.tile([TPC * GR, C * RW], F32, tag="ydall", bufs=4)
                for t in range(TPC):
                    nc.sync.dma_start(
                        ydall[t * GR : (t + 1) * GR, :],
                        bass.AP(
                            tensor=ypd,
                            offset=t * C * YL + g * GR,
                            ap=[[1, GR], [YL, C], [1, RW]],
                        ),
                    )
                ydalls.append(ydall)
            for g in range(NG):
                ydall = ydalls[g]
                acc = pa.tile([TPC * GR, RW], F32, tag="acc")
                for c in range(C):
                    ydc = ydall[:, c * RW : (c + 1) * RW]
                    bc = xneg[:, g * C + c : g * C + c + 1]
                    if c == 0:
                        nc.scalar.activation(acc[:], ydc, AF.Square, bias=bc)
                    else:
                        sq = pa.tile([TPC * GR, RW], F32, tag="sq", bufs=3)
                        nc.scalar.activation(sq[:], ydc, AF.Square, bias=bc)
                        nc.gpsimd.tensor_add(acc[:], acc[:], sq[:])
                dall = pa.tile([TPC * GR, RW], F32, tag="dall")
                nc.scalar.activation(dall[:], acc[:], AF.Sqrt)
                # relayout: trace t's rows -> partition t
                for t in range(TPC):
                    nc.scalar.dma_start(
                        dpband[t : t + 1, g * GR : (g + 1) * GR, 0:RW],
                        dall[t * GR : (t + 1) * GR, :],
                    )

            # ---------------- Phase B: the serial DP ---------------------
            # row 1 reads the row-900 seed band straight out of dpband.
            for r in range(1, K):
                i = I0 + r
                p = dpband[0:TPC, 0, 0:SW] if r == 1 else prev[0:TPC, 0:SW]
                # real band cells: u in [0, L); L shrinks once i+100 > 1023.
                L = RW if i <= 1124 - RW else 1124 - i
                # m[u] = min(prev[u], prev[u+1]); for full rows m[199] is the
                # preset 0 (prev[200] is the boundary); once rows trim, the
                # last real cell needs the explicit min with prev[L].
                LT = L - 1 if i <= 923 else L
                nc.vector.tensor_tensor(
                    m[0:TPC, 0:LT], p[:, 0:LT], p[:, 1 : LT + 1], OP.min
                )
                nc.vector.tensor_tensor_scan(
                    cur[0:TPC, 0:L],
                    m[0:TPC, 0:L],
                    dpband[0:TPC, r, 0:L],
                    0.0,
                    op0=OP.min,
                    op1=OP.add,
                )
                prev, cur = cur, prev

            nc.sync.dma_start(out[:, :], prev[0:TPC, WIN : WIN + 1])
    if not nc.is_finalized():
        nc.finalize()  # runs Bacc.compile(): wait-splitting + reg alloc
    return nc


def _shard_inputs(x, y):
    """x, y: (T, N, C) full -> per-core input maps."""
    xt = x.transpose(1, 0, 2)                              # (N, T, C)
    yt = y.transpose(1, 0, 2)
    # (N, K, C) -> per-core [TPC*GR, NG*C] with [t*GR+r, g*C+c] = x[t, g*GR+r, c]
    xs = np.ascontiguousarray(xt[:, I0:T, :], dtype=np.float32)
    xs = xs.reshape(N, NG, GR, C).transpose(0, 2, 1, 3).reshape(N, GR, NG * C)
    ypd = np.zeros((N, C, YL), dtype=np.float32)
    ypd[:, :, 0 : T - J0] = yt[:, J0:T, :].transpose(0, 2, 1)
    in_maps = []
    for k in range(NCORES):
        sl = slice(k * TPC, (k + 1) * TPC)
        in_maps.append(
            {
                "x": np.ascontiguousarray(
                    xs[sl].reshape(TPC * GR, NG * C)
                ),
                "ypd": np.ascontiguousarray(ypd[sl]),
            }
        )
    return in_maps


LAST_RESULTS = None


def kernel(x, y, _trace=False):
    global LAST_RESULTS
    if "nc" not in _CACHE:
        _CACHE["nc"] = _build_nc()
    nc = _CACHE["nc"]
    in_maps = _shard_inputs(np.asarray(x), np.asarray(y))
    res = run_bass_kernel_spmd(
        nc, in_maps, list(range(NCORES)), trace=_trace
    )
    LAST_RESULTS = res
    vals = np.concatenate([r["out"].reshape(-1) for r in res.results])
    return np.float32(vals.astype(np.float32).sum() / np.float32(N))


# revision 9
# speedup vs baseline: 1.1056x; 1.1056x over previous
"""Banded DTW (window=100) on Trainium2, 8 NeuronCores — truncated-DP version.

Problem: x, y of shape (T=1024, N=32, C=4). Per trace n: banded DTW on the
(1024, 1024) pairwise-distance grid, band j in [i-100, i+100); cells outside
the band hold 0 (torch quirk); row 0 / col 0 seeded with raw distances.
Output: scalar mean over the 32 per-trace DTW values.

Key optimization: the out-of-band zeros leak into the band at BOTH band edges
(acc[i, i+99] = d, and the row state re-enters at 0 on the left edge), so the
DP forgets its history: a monotone lower/upper-bound sandwich (init row i0
with 0s vs +BIG) shows the final cell is exact for any i0 <= 900. We run only
rows 900..1023 (124 rows instead of 1024), seeding row 900 with its raw
distance band — certified rel err ~1e-7 in fp64 (fp16 DP state was tried
and fails: DP values ~200-600 make fp16 rounding accumulate to 2.8e-2).

Layout (4 traces per core, data parallel over 8 cores):
  Band-relative storage u = j - (i - 100), u in [0, 200); column 200 is a
  never-written zero boundary slot (replaces the baseline's mask multiply).
  Row recurrence  cur[u] = min(min(prev[u], prev[u+1]), cur[u-1]) + d[u]
  = ONE tensor_tensor (min of shifted pair) + ONE tensor_tensor_scan
  (op0=min, op1=add) per row, all fp32,
  4 traces riding the partition dim. Phase A computes banded distances for
  all 4 traces at once on 124 partitions (4 traces x 31 rows per group) and
  DMA-relayouts each trace's rows into its DP partition. Startup hiding:
  all bulk DMAs ride the ACT HWDGE ring, xneg on GPSIMD, a warmup Sqrt
  hoists the ACT table load, and the first DP row reads the seed band
  straight out of dpband (no seed copy).
"""

import os
import sys

import numpy as np

for _p in ("/opt/trn_rl_repo", "/root/.axon_site/_ro/trn_rl_repo"):
    if os.path.isdir(_p) and _p not in sys.path:
        sys.path.insert(0, _p)

import concourse.bass as bass
import concourse.bacc as bacc
import concourse.mybir as mybir
from concourse.bass_utils import run_bass_kernel_spmd
from concourse.tile import TileContext

T = 1024          # time steps (both sequences)
C = 4             # channels
N = 32            # traces
NCORES = 8
TPC = N // NCORES  # 4 traces per core
WIN = 100
I0 = 900           # first DP row (certified: any i0 <= 900 is exact)
K = T - I0         # 124 DP rows
RW = 2 * WIN       # 200 real band cells per row, u in [0, 200)
SW = RW + 1        # row stride: +1 zero boundary slot (u=200)
GR = 31            # phase-A rows per group (4 traces x 31 rows = 124 parts)
NG = K // GR       # 4 groups
J0 = I0 - WIN      # 800: first y index needed
YL = 324           # y slice length: j in [800, 1124), zero-padded past 1023

F32 = mybir.dt.float32
F16 = mybir.dt.float16
AF = mybir.ActivationFunctionType
OP = mybir.AluOpType

_CACHE = {}


def _build_nc():
    # Bacc (not raw Bass): its compile() pass splits multi-wait sync infos —
    # the TRN2 ISA allows at most one sync wait per instruction.
    nc = bacc.Bacc()
    # x pre-arranged on host to the phase-A layout: [t*GR+r, g*C+c]
    x = nc.declare_dram_parameter("x", [TPC * GR, NG * C], F32, isOutput=False)
    ypd = nc.declare_dram_parameter("ypd", [TPC, C, YL], F32, isOutput=False)
    out = nc.declare_dram_parameter("out", [TPC, 1], F32, isOutput=True)

    with TileContext(nc) as tc:
        with (
            tc.tile_pool(name="pa", bufs=2) as pa,
            tc.tile_pool(name="dp", bufs=1) as dp,
        ):
            # warmup: force the Square/Sqrt ACT table load before any data
            # lands, off the group-0 critical path.
            warm = dp.tile([1, 1], F32)
            nc.gpsimd.memset(warm[:], 1.0)
            nc.scalar.activation(warm[:], warm[:], AF.Sqrt)

            # DP-state tiles + memsets early so the Pool queue clears them
            # while inputs stream in.
            dpband = dp.tile([TPC, K, SW], F32)
            # zero the boundary column (u=200): read as prev[u+1] at u=199
            # and as the i=924 row's prev[200]; never written afterwards.
            nc.gpsimd.memset(dpband[0:TPC, 0:K, RW:SW], 0.0)
            prev = dp.tile([TPC, SW], F32)
            cur = dp.tile([TPC, SW], F32)
            m = dp.tile([TPC, SW], F32)
            nc.gpsimd.memset(m[:], 0.0)    # m[199] stays 0 for full rows
            nc.gpsimd.memset(prev[:], 0.0)
            nc.gpsimd.memset(cur[:], 0.0)  # cur[200] stays 0 forever

            # x for all groups in one contiguous DMA (host pre-arranged)
            xall = pa.tile([TPC * GR, NG * C], F32, tag="xall")
            nc.scalar.dma_start(xall[:], x[:, :])
            xneg = pa.tile([TPC * GR, NG * C], F32, tag="xneg")
            nc.gpsimd.tensor_scalar_mul(xneg[:], xall[:], -1.0)

            # ---------------- Phase A: banded distances -----------------
            # group g covers rows I0+g*GR .. +GR of ALL 4 traces:
            # partition p = t*GR + r. D[p][u] = ||x[t,row] - y[row-100+u]||.
            # sq_c = (y_c - x_c)^2 via ACT Square with per-partition bias
            # (exact); adds on GPSIMD; DVE stays free for the DP chain.
            # All DMAs ride the ACT HWDGE ring (~600ns issue, async
            # transfer on parallel queues). SP's software-DGE path occupies
            # the sequencer for the whole descriptor build (measured 4-6us
            # per patterned DMA) — never put bulk DMAs there.
            for g in range(NG):
                # ydall[t*GR+r, c*RW+u] = ypd[t, c, g*GR + r + u]
                ydall = pa.tile([TPC * GR, C * RW], F32, tag="ydall", bufs=2)
                for t in range(TPC):
                    nc.scalar.dma_start(
                        ydall[t * GR : (t + 1) * GR, :],
                        bass.AP(
                            tensor=ypd,
                            offset=t * C * YL + g * GR,
                            ap=[[1, GR], [YL, C], [1, RW]],
                        ),
                    )
                acc = pa.tile([TPC * GR, RW], F32, tag="acc")
                for c in range(C):
                    ydc = ydall[:, c * RW : (c + 1) * RW]
                    bc = xneg[:, g * C + c : g * C + c + 1]
                    if c == 0:
                        nc.scalar.activation(acc[:], ydc, AF.Square, bias=bc)
                    else:
                        sq = pa.tile([TPC * GR, RW], F32, tag="sq", bufs=3)
                        nc.scalar.activation(sq[:], ydc, AF.Square, bias=bc)
                        nc.gpsimd.tensor_add(acc[:], acc[:], sq[:])
                dall = pa.tile([TPC * GR, RW], F32, tag="dall")
                nc.scalar.activation(dall[:], acc[:], AF.Sqrt)
                # relayout: trace t's rows -> partition t
                for t in range(TPC):
                    nc.scalar.dma_start(
                        dpband[t : t + 1, g * GR : (g + 1) * GR, 0:RW],
                        dall[t * GR : (t + 1) * GR, :],
                    )

            # ---------------- Phase B: the serial DP ---------------------
            # row 1 reads the row-900 seed band straight out of dpband.
            for r in range(1, K):
                i = I0 + r
                p = dpband[0:TPC, 0, 0:SW] if r == 1 else prev[0:TPC, 0:SW]
                # real band cells: u in [0, L); L shrinks once i+100 > 1023.
                L = RW if i <= 1124 - RW else 1124 - i
                # m[u] = min(prev[u], prev[u+1]); for full rows m[199] is the
                # preset 0 (prev[200] is the boundary); once rows trim, the
                # last real cell needs the explicit min with prev[L].
                LT = L - 1 if i <= 923 else L
                nc.vector.tensor_tensor(
                    m[0:TPC, 0:LT], p[:, 0:LT], p[:, 1 : LT + 1], OP.min
                )
                nc.vector.tensor_tensor_scan(
                    cur[0:TPC, 0:L],
                    m[0:TPC, 0:L],
                    dpband[0:TPC, r, 0:L],
                    0.0,
                    op0=OP.min,
                    op1=OP.add,
                )
                prev, cur = cur, prev

            nc.sync.dma_start(out[:, :], prev[0:TPC, WIN : WIN + 1])
    if not nc.is_finalized():
        nc.finalize()  # runs Bacc.compile(): wait-splitting + reg alloc
    return nc


def _shard_inputs(x, y):
    """x, y: (T, N, C) full -> per-core input maps."""
    xt = x.transpose(1, 0, 2)                              # (N, T, C)
    yt = y.transpose(1, 0, 2)
    # (N, K, C) -> per-core [TPC*GR, NG*C] with [t*GR+r, g*C+c] = x[t, g*GR+r, c]
    xs = np.ascontiguousarray(xt[:, I0:T, :], dtype=np.float32)
    xs = xs.reshape(N, NG, GR, C).transpose(0, 2, 1, 3).reshape(N, GR, NG * C)
    ypd = np.zeros((N, C, YL), dtype=np.float32)
    ypd[:, :, 0 : T - J0] = yt[:, J0:T, :].transpose(0, 2, 1)
    in_maps = []
    for k in range(NCORES):
        sl = slice(k * TPC, (k + 1) * TPC)
        in_maps.append(
            {
                "x": np.ascontiguousarray(
                    xs[sl].reshape(TPC * GR, NG * C)
                ),
                "ypd": np.ascontiguousarray(ypd[sl]),
            }
        )
    return in_maps


LAST_RESULTS = None


def kernel(x, y, _trace=False):
    global LAST_RESULTS
    if "nc" not in _CACHE:
        _CACHE["nc"] = _build_nc()
    nc = _CACHE["nc"]
    in_maps = _shard_inputs(np.asarray(x), np.asarray(y))
    res = run_bass_kernel_spmd(
        nc, in_maps, list(range(NCORES)), trace=_trace
    )
    LAST_RESULTS = res
    vals = np.concatenate([r["out"].reshape(-1) for r in res.results])
    return np.float32(vals.astype(np.float32).sum() / np.float32(N))
